# revision 8
# baseline (speedup 1.0000x reference)
import os
import sys
import tempfile

sys.path.insert(0, "/opt/trn_rl_repo")

from contextlib import ExitStack

import numpy as np

import concourse.bass as bass
import concourse.bacc as bacc
import concourse.mybir as mybir
import concourse.tile as tile
from concourse import library_config
from concourse.bass_utils import run_bass_kernel_spmd

N = 50000
E = 1600000
IN = 128
H = 4
C = 16
HC = H * C
P = 128
NCORES = 8
NT = 50                      # dst tiles per core (tail tiles partly fake)
NPC_PAD = NT * P             # 6400 dst slots per core
NX = NCORES * NPC_PAD        # 51200 dst slots
NSH = 6656                   # projected nodes per core (6400 real + 256 pad)
NGS = NSH // 512             # 13 projection groups per core
NXT = NCORES * NSH           # 53248 gather-table rows
RW = 128                     # table row: xp[64]|a_src[4]|a_dst[4]|pad (bf16)
W1LIM = 32768                # window 1 = rows [0, 32768)
W2OFF = NXT - 32768          # window 2 = rows [20480, 53248)
GCHUNK = int(os.environ.get("GAT_GCHUNK", "8"))   # rows per dma_gather call

_cache = {}
_last_results = None
_ABL = set(os.environ.get("GAT_ABL", "").split(",")) - {""}


def build_program(k1s, k2s):
    f32 = mybir.dt.float32
    bf16 = mybir.dt.bfloat16
    i16 = mybir.dt.int16
    X = mybir.AxisListType.X
    mult = mybir.AluOpType.mult
    add = mybir.AluOpType.add
    sw1 = [0] + list(np.cumsum([8 * k for k in k1s]))
    sw2 = [0] + list(np.cumsum([8 * k for k in k2s]))

    nc = bacc.Bacc(None, target_bir_lowering=False, debug=False,
                   dynamic_dma_scratch_size=65536, num_devices=NCORES)
    xs_ext = nc.declare_dram_parameter("xs", [IN, NSH], bf16, isOutput=False)
    wr_ext = nc.declare_dram_parameter("wrhs", [IN, RW], bf16, isOutput=False)
    bias_ext = nc.declare_dram_parameter("bias", [P, HC], f32, isOutput=False)
    # unreplicated index tables: 16 rows, replicated to 128 on device
    ix1_ext = nc.declare_dram_parameter("ix1", [16, sw1[-1]], i16,
                                        isOutput=False)
    ix2_ext = nc.declare_dram_parameter("ix2", [16, sw2[-1]], i16,
                                        isOutput=False)
    # own-row window masks (1.0 where own row lives in that window)
    m1_ext = nc.declare_dram_parameter("m1", [P, NT * H], bf16, isOutput=False)
    m2_ext = nc.declare_dram_parameter("m2", [P, NT * H], bf16, isOutput=False)
    out_ext = nc.declare_dram_parameter("out", [NPC_PAD, HC], bf16,
                                        isOutput=True)
    shardbuf = nc.dram_tensor("shardbuf", [NSH, 72], bf16)
    gathered = nc.dram_tensor("gathered", [NXT, 72], bf16)
    xpb = nc.dram_tensor("xpb", [NXT, RW], bf16)
    # dma_gather mis-addresses offset source views, so window 2 gets its own
    # physical copy of rows [W2OFF, NXT)
    xpb2 = nc.dram_tensor("xpb2", [NXT - W2OFF, RW], bf16)

    with tile.TileContext(nc) as tc, ExitStack() as ctx:
        singles = ctx.enter_context(tc.tile_pool(name="singles", bufs=1))
        nc.gpsimd.load_library(library_config.mlp)
        wr_sb = singles.tile([IN, RW], bf16)
        nc.sync.dma_start(out=wr_sb[:], in_=wr_ext[:])
        bias_sb = singles.tile([P, HC], f32)
        nc.sync.dma_start(out=bias_sb[:], in_=bias_ext[:])
        ix1_sb = singles.tile([P, sw1[-1]], i16)
        ix2_sb = singles.tile([P, sw2[-1]], i16)
        for g in range(8):
            nc.sync.dma_start(out=ix1_sb[16 * g:16 * (g + 1), :],
                              in_=ix1_ext[:])
            nc.sync.dma_start(out=ix2_sb[16 * g:16 * (g + 1), :],
                              in_=ix2_ext[:])
        m1_sb = singles.tile([P, NT * H], bf16)
        nc.sync.dma_start(out=m1_sb[:], in_=m1_ext[:])
        m2_sb = singles.tile([P, NT * H], bf16)
        nc.sync.dma_start(out=m2_sb[:], in_=m2_ext[:])
        neg_sb = singles.tile([1, 4], bf16)
        nc.vector.memset(neg_sb[:], -1e30)

        # phase 1: project local shard -> [xp | a_src | a_dst] rows
        # (local newid g*512+b*128+p -> shardbuf row g*512+p*4+b)
        with ExitStack() as p1:
            xbufs = p1.enter_context(tc.tile_pool(name="xbufs", bufs=3))
            psums = p1.enter_context(tc.tile_pool(name="psums", bufs=4,
                                                  space="PSUM"))
            obufs = p1.enter_context(tc.tile_pool(name="obufs", bufs=3))
            for g in range(NGS if "noproj" not in _ABL else 0):
                c0 = g * 512
                xtile = xbufs.tile([IN, 512], bf16)
                nc.sync.dma_start(out=xtile[:], in_=xs_ext[:, c0:c0 + 512])
                ps = psums.tile([P, 4 * RW], f32, space="PSUM")
                for b in range(4):
                    nc.tensor.matmul(out=ps[:, b * RW:(b + 1) * RW],
                                     lhsT=xtile[:, b * P:(b + 1) * P],
                                     rhs=wr_sb[:], start=True, stop=True)
                xa = obufs.tile([P, 4 * RW], bf16)
                nc.scalar.copy(out=xa[:], in_=ps[:])
                xa3 = xa[:].rearrange("p (q w) -> p q w", q=4, w=RW)[:, :, 0:72]
                nc.sync.dma_start(
                    out=shardbuf[c0:c0 + 512, :].rearrange(
                        "(p q) w -> p q w", p=P, q=4),
                    in_=xa3)

        # all-gather shards -> full compact table, then spread to 256B rows
        nc.gpsimd.collective_compute(
            "AllGather", mybir.AluOpType.bypass,
            replica_groups=[list(range(NCORES))],
            ins=[shardbuf[:]], outs=[gathered[:]])
        nc.sync.dma_start(out=xpb[:, 0:72], in_=gathered[:])
        nc.sync.dma_start(out=xpb2[:, 0:72], in_=gathered[W2OFF:, :])
        nc.sync.dma_start(out=xpb[NEG1:NEG1 + 1, 64:68], in_=neg_sb[:])
        nc.sync.dma_start(out=xpb[NEG2:NEG2 + 1, 64:68], in_=neg_sb[:])
        nc.sync.dma_start(out=xpb2[NEG2 - W2OFF:NEG2 - W2OFF + 1, 64:68],
                          in_=neg_sb[:])

        # phase 2
        gath = ctx.enter_context(tc.tile_pool(name="gath", bufs=3))
        work = ctx.enter_context(tc.tile_pool(name="work", bufs=2))
        small = ctx.enter_context(tc.tile_pool(name="small", bufs=3))
        for t in range(NT):
            K1, K2 = k1s[t], k2s[t]
            E1, E2 = K1 - 1, K2 - 1   # edge slots (slot 0 = own row)
            D = E1 + E2
            r0 = t * P

            def chunked_gather(xg, src, isb, o0, k):
                for c0c in range(0, k, GCHUNK):
                    kc = min(GCHUNK, k - c0c)
                    nc.gpsimd.dma_gather(
                        xg[:, c0c * RW:(c0c + kc) * RW].rearrange(
                            "p (j w) -> p j w", j=kc, w=RW),
                        src[:], isb[:, o0 + 8 * c0c:o0 + 8 * (c0c + kc)],
                        kc * P, kc * P, RW)

            xg1 = gath.tile([P, K1 * RW], bf16)
            chunked_gather(xg1[:], xpb, ix1_sb, sw1[t], K1)
            xg2 = gath.tile([P, K2 * RW], bf16)
            chunked_gather(xg2[:], xpb2, ix2_sb, sw2[t], K2)
            x13 = xg1[:].rearrange("p (j w) -> p j w", j=K1, w=RW)
            x23 = xg2[:].rearrange("p (j w) -> p j w", j=K2, w=RW)

            if "nocompute" in _ABL:
                outsb0 = small.tile([P, HC], bf16)
                nc.vector.tensor_copy(out=outsb0[:], in_=xg1[:, :HC])
                nc.sync.dma_start(out=out_ext[r0:r0 + P, :], in_=outsb0[:])
                continue

            # a_dst of the tile's own nodes: slot 0 of whichever window
            adst = small.tile([P, H], f32)
            nc.vector.tensor_tensor(out=adst[:], in0=xg1[:, 68:72],
                                    in1=m1_sb[:, t * H:(t + 1) * H], op=mult)
            adst2t = small.tile([P, H], f32)
            nc.vector.tensor_tensor(out=adst2t[:], in0=xg2[:, 68:72],
                                    in1=m2_sb[:, t * H:(t + 1) * H], op=mult)
            nc.vector.tensor_tensor(out=adst[:], in0=adst[:], in1=adst2t[:],
                                    op=add)

            e = small.tile([P, D * H], f32)
            e3 = e[:].rearrange("p (d h) -> p d h", d=D, h=H)
            adst_b = adst[:].rearrange("p (one h) -> p one h", one=1)
            nc.vector.tensor_tensor(out=e3[:, :E1, :],
                                    in0=x13[:, 1:, 64:68],
                                    in1=adst_b.to_broadcast([P, E1, H]),
                                    op=add)
            nc.vector.tensor_tensor(out=e3[:, E1:, :],
                                    in0=x23[:, 1:, 64:68],
                                    in1=adst_b.to_broadcast([P, E2, H]),
                                    op=add)
            nc.vector.scalar_tensor_tensor(out=e[:], in0=e[:], scalar=0.2,
                                           in1=e[:], op0=mult,
                                           op1=mybir.AluOpType.max)
            exb = work.tile([P, D * HC], bf16)
            exb4 = exb[:].rearrange("p (d h c) -> p d h c", d=D, h=H, c=C)
            e4b = e[:].rearrange("p (d h one) -> p d h one", d=D, h=H,
                                 one=1).to_broadcast([P, D, H, C])
            nc.scalar.activation(out=exb4, in_=e4b,
                                 func=mybir.ActivationFunctionType.Exp)
            s = small.tile([P, H], f32)
            nc.vector.tensor_reduce(
                out=s[:],
                in_=exb[:].rearrange("p (d h c) -> p h c d", d=D, h=H,
                                     c=C)[:, :, 0:1, :],
                axis=X, op=add)
            sinv = small.tile([P, H], f32)
            nc.vector.reciprocal(out=sinv[:], in_=s[:])

            msg = work.tile([P, D * HC], bf16)
            msg3 = msg[:].rearrange("p (d w) -> p d w", d=D, w=HC)
            exb3 = exb[:].rearrange("p (d w) -> p d w", d=D, w=HC)
            nc.vector.tensor_tensor(out=msg3[:, :E1, :],
                                    in0=x13[:, 1:, 0:64],
                                    in1=exb3[:, :E1, :], op=mult)
            nc.vector.tensor_tensor(out=msg3[:, E1:, :],
                                    in0=x23[:, 1:, 0:64],
                                    in1=exb3[:, E1:, :], op=mult)
            n = D
            while n > 1:
                if n % 2:
                    nc.vector.tensor_tensor(out=msg3[:, n - 2, :],
                                            in0=msg3[:, n - 2, :],
                                            in1=msg3[:, n - 1, :], op=add)
                    n -= 1
                h = n // 2
                nc.vector.tensor_tensor(out=msg[:, :h * HC],
                                        in0=msg[:, :h * HC],
                                        in1=msg[:, h * HC:2 * h * HC], op=add)
                n = h

            outsb = small.tile([P, HC], f32)
            sinv_b = sinv[:].rearrange("p (h one) -> p h one",
                                       h=H, one=1).to_broadcast([P, H, C])
            nc.vector.tensor_tensor(
                out=outsb[:].rearrange("p (h c) -> p h c", h=H, c=C),
                in0=msg3[:, 0, :].rearrange("p (h c) -> p h c", h=H, c=C),
                in1=sinv_b, op=mult)
            nc.vector.tensor_tensor(out=outsb[:], in0=outsb[:], in1=bias_sb[:],
                                    op=add)
            outbf = small.tile([P, HC], bf16)
            nc.vector.tensor_copy(out=outbf[:], in_=outsb[:])
            nc.sync.dma_start(out=out_ext[r0:r0 + P, :], in_=outbf[:])

    nc.compile()
    return nc


def _trow(rank):
    """T-rank -> gather-table row (core-contiguous, phase-1 interleave)."""
    rank = np.asarray(rank)
    ct, l = rank // 6400, rank % 6400
    return ct * NSH + (l // 512) * 512 + (l % P) * 4 + ((l // P) % 4)


def _padrow(core, j):
    """Row of pad slot j (0..255) of a core (content = projected zeros)."""
    l = 6400 + j
    return core * NSH + (l // 512) * 512 + (l % P) * 4 + ((l // P) % 4)


ZERO1 = int(_padrow(0, 0))
NEG1 = int(_padrow(0, 1))
ZERO2 = int(_padrow(7, 0))
NEG2 = int(_padrow(7, 1))
assert ZERO1 < W1LIM and NEG1 < W1LIM
assert ZERO2 >= W2OFF and NEG2 >= W2OFF and NEG2 - W2OFF < 32768


def _wrap16(vals, k):
    """vals[p, d] -> unreplicated int16 [16, 8k]: unwrapped[i]=w[i%16,i//16],
    i = d*128 + p (device replicates rows 0:16 to all 8 groups)."""
    i = np.arange(P * k)
    w = np.zeros((16, 8 * k), np.int16)
    w[i % 16, i // 16] = vals[i % P, i // P]
    return w


def _preprocess(edge_index):
    src = edge_index[0].astype(np.int64)
    dst = edge_index[1].astype(np.int64)
    deg = np.bincount(dst, minlength=N)

    # T mapping: rank = identity order, core-contiguous blocks of 6400
    orderT = np.arange(N)
    trank_of_node = np.arange(N)
    trow_of_node = _trow(trank_of_node)

    srcr = trow_of_node[src]

    # per-dst window counts over source rows
    n1 = np.zeros(N, np.int64)
    n2 = np.zeros(N, np.int64)
    m = np.zeros(N, np.int64)
    c1 = srcr < W2OFF
    c2 = srcr >= W1LIM
    np.add.at(n1, dst[c1], 1)
    np.add.at(n2, dst[c2], 1)
    np.add.at(m, dst[~c1 & ~c2], 1)
    a = np.clip((n2 + m - n1 + 1) // 2, 0, m)   # overlap going to window 1
    A = n1 + a + 1 + (deg == 0)   # + own slot (+ zero-dummy for empty nodes)
    B = n2 + m - a + 1

    # S mapping (dst shard slots): sort by (A, B) desc -> uniform tiles;
    # snake B within A-groups so tile-range B-maxes stay tight at boundaries
    orderS = np.lexsort((B, A))[::-1]
    As = A[orderS]
    gid = np.cumsum(np.r_[True, As[1:] != As[:-1]]) - 1
    for g in range(gid.max() + 1):
        if g % 2 == 1:
            sel = np.where(gid == g)[0]
            orderS[sel] = orderS[sel[::-1]]
    r = np.arange(NX)
    k, p = r // P, r % P
    snewid_of_rank = (k % NCORES) * NPC_PAD + (k // NCORES) * P + p
    snewid_of_node = np.empty(N, np.int64)
    snewid_of_node[orderS] = snewid_of_rank[:N]
    node_of_snewid = np.full(NX, -1, np.int64)
    node_of_snewid[snewid_of_rank[:N]] = orderS

    # per-tile K maxes, shared across cores for the SPMD program
    tile_of = (np.arange(NX) % NPC_PAD) // P
    Av = np.zeros(NX, np.int64)
    Bv = np.zeros(NX, np.int64)
    Av[snewid_of_node] = A
    Bv[snewid_of_node] = B
    k1s, k2s = [], []
    for t in range(NT):
        sel = tile_of == t
        k1s.append(max(2, int(Av[sel].max())))
        k2s.append(max(2, int(Bv[sel].max())))

    # slot tables indexed by snewid; slot 0 = own row (or dummy)
    ix1 = np.full((NX, max(k1s)), NEG1, np.int32)
    ix2 = np.full((NX, max(k2s)), NEG2 - W2OFF, np.int32)
    sn = snewid_of_node
    own_r = trow_of_node
    own_w1 = own_r < W1LIM
    ix1[sn[own_w1], 0] = own_r[own_w1]
    ix2[sn[~own_w1], 0] = own_r[~own_w1] - W2OFF
    m1 = np.zeros(NX, np.float32)
    m1[sn[own_w1]] = 1.0

    # order each node's edges: window-1 = n1 edges then a of overlap
    dstn = sn[dst]
    osort = np.argsort(dstn, kind="stable")
    dstn_s, srcr_s = dstn[osort], srcr[osort]
    cls = np.where(c1[osort], 0,
                   np.where(c2[osort], 2, 1))  # 0=w1only, 1=overlap, 2=w2only
    keys = dstn_s * 4 + cls
    ordc = np.argsort(keys, kind="stable")
    keys_s = keys[ordc]
    first = np.r_[True, keys_s[1:] != keys_s[:-1]]
    grp_start = np.zeros(len(keys_s), np.int64)
    grp_start[first] = np.arange(len(keys_s))[first]
    grp_start = np.maximum.accumulate(grp_start)
    pos_in_cls = np.arange(len(keys_s)) - grp_start
    posc = np.empty(E, np.int64)
    posc[ordc] = pos_in_cls
    dn = dstn_s
    node_dn = node_of_snewid[dn]
    av = a[node_dn]
    n1v = n1[node_dn]
    mv = m[node_dn]
    # window-1 slots (1-based, after own): w1only edges then first av overlap
    w1_mask = (cls == 0) | ((cls == 1) & (posc < av))
    slot_w1 = 1 + np.where(cls == 0, posc, n1v + posc)
    ix1[dn[w1_mask], slot_w1[w1_mask]] = srcr_s[w1_mask]
    ovr = (cls == 1) & (posc >= av)
    ix2[dn[ovr], 1 + posc[ovr] - av[ovr]] = srcr_s[ovr] - W2OFF
    mw2 = cls == 2
    ix2[dn[mw2], 1 + (mv - av)[mw2] + posc[mw2]] = srcr_s[mw2] - W2OFF
    # empty nodes: one zero-dummy edge slot in window 1
    empty_sn = sn[deg == 0]
    ix1[empty_sn, 1] = ZERO1

    return (k1s, k2s, ix1, ix2, m1, orderT, node_of_snewid)


def _prepare(x, edge_index, W, att_src, att_dst, bias):
    x = np.asarray(x, np.float32)
    edge_index = np.asarray(edge_index)
    W = np.asarray(W, np.float32)
    att_src = np.asarray(att_src, np.float32)
    att_dst = np.asarray(att_dst, np.float32)
    bias = np.asarray(bias, np.float32)

    (k1s, k2s, ix1, ix2, m1, orderT, node_of_snewid) = _preprocess(edge_index)
    key = (tuple(k1s), tuple(k2s))
    if key not in _cache:
        _cache[key] = build_program(k1s, k2s)
    nc = _cache[key]

    wa_src = np.einsum("ihc,hc->ih", W.T.reshape(IN, H, C), att_src)
    wa_dst = np.einsum("ihc,hc->ih", W.T.reshape(IN, H, C), att_dst)
    wrhs = np.zeros((IN, RW), np.float32)
    wrhs[:, :HC] = W.T
    wrhs[:, HC:HC + H] = wa_src
    wrhs[:, HC + H:HC + 2 * H] = wa_dst
    bias_rep = np.tile(bias.reshape(1, HC), (P, 1)).astype(np.float32)

    import ml_dtypes

    def bf16(arr):
        return arr.astype(ml_dtypes.bfloat16)

    xT = np.ascontiguousarray(x.T)  # [IN, N], T order = identity
    in_maps = []
    for c in range(NCORES):
        lo = c * NPC_PAD
        w1 = [_wrap16(ix1[lo + t * P:lo + (t + 1) * P, :k1s[t]]
                      .astype(np.int16), k1s[t]) for t in range(NT)]
        w2 = [_wrap16(ix2[lo + t * P:lo + (t + 1) * P, :k2s[t]]
                      .astype(np.int16), k2s[t]) for t in range(NT)]
        m1c = m1[lo:lo + NPC_PAD].reshape(NT, P).T          # [P, NT]
        m1h = np.repeat(m1c[:, :, None], H, axis=2).reshape(P, NT * H)
        hi = min((c + 1) * 6400, N)
        xs = np.zeros((IN, NSH), np.float32)
        xs[:, :hi - c * 6400] = xT[:, c * 6400:hi]
        in_maps.append({
            "xs": bf16(xs), "wrhs": bf16(wrhs), "bias": bias_rep,
            "ix1": np.concatenate(w1, axis=1),
            "ix2": np.concatenate(w2, axis=1),
            "m1": bf16(m1h), "m2": bf16(1.0 - m1h),
        })

    def post(res):
        arr = np.concatenate([np.asarray(res[c]["out"], np.float32)
                              for c in range(NCORES)], axis=0)
        valid = node_of_snewid >= 0
        out = np.zeros((N, HC), np.float32)
        out[node_of_snewid[valid]] = arr[valid]
        return out

    return nc, in_maps, post


def kernel(x, edge_index, W, att_src, att_dst, bias):
    nc, in_maps, post = _prepare(x, edge_index, W, att_src, att_dst, bias)
    global _last_results
    tmpdir = None
    if os.environ.get("BASS_TRACE"):
        tmpdir = tempfile.mkdtemp(prefix="gat_trace_")
    _last_results = run_bass_kernel_spmd(nc, in_maps, list(range(NCORES)),
                                         tmpdir=tmpdir)
    return post(_last_results.results)


# revision 13
# speedup vs baseline: 1.9456x; 1.9456x over previous
import os
import sys
import tempfile

sys.path.insert(0, "/opt/trn_rl_repo")

from contextlib import ExitStack

import numpy as np

import concourse.bass as bass
import concourse.bacc as bacc
import concourse.mybir as mybir
import concourse.tile as tile
from concourse import library_config
from concourse.bass_utils import run_bass_kernel_spmd

N = 50000
E = 1600000
IN = 128
H = 4
C = 16
HC = H * C
P = 128
NCORES = 8
NT = 50                      # dst tiles per core (tail tiles partly fake)
NPC_PAD = NT * P             # 6400 dst slots per core
NX = NCORES * NPC_PAD        # 51200 dst slots
NSH = 6656                   # projected nodes per core (6400 real + 256 pad)
NGS = NSH // 512             # 13 projection groups per core
NXT = NCORES * NSH           # 53248 gather-table rows
RW = 128                     # table row: xp[64]|a_src[4]|a_dst[4]|pad (bf16)
W1LIM = 32768                # window 1 = rows [0, 32768)
W2OFF = NXT - 32768          # window 2 = rows [20480, 53248)
GCHUNK = int(os.environ.get("GAT_GCHUNK", "8"))   # rows per dma_gather call

_cache = {}
_last_results = None
_ABL = set(os.environ.get("GAT_ABL", "").split(",")) - {""}


def build_program(k1s, k2s):
    f32 = mybir.dt.float32
    bf16 = mybir.dt.bfloat16
    i16 = mybir.dt.int16
    X = mybir.AxisListType.X
    mult = mybir.AluOpType.mult
    add = mybir.AluOpType.add
    sw1 = [0] + list(np.cumsum([8 * k for k in k1s]))
    sw2 = [0] + list(np.cumsum([8 * k for k in k2s]))

    nc = bacc.Bacc(None, target_bir_lowering=False, debug=False,
                   dynamic_dma_scratch_size=65536, num_devices=NCORES)
    xs_ext = nc.declare_dram_parameter("xs", [IN, NSH], bf16, isOutput=False)
    wr_ext = nc.declare_dram_parameter("wrhs", [IN, RW], bf16, isOutput=False)
    bias_ext = nc.declare_dram_parameter("bias", [P, HC], f32, isOutput=False)
    # unreplicated index tables: 16 rows, replicated to 128 on device
    ix1_ext = nc.declare_dram_parameter("ix1", [16, sw1[-1]], i16,
                                        isOutput=False)
    ix2_ext = nc.declare_dram_parameter("ix2", [16, sw2[-1]], i16,
                                        isOutput=False)
    # own-row window masks (1.0 where own row lives in that window)
    m1_ext = nc.declare_dram_parameter("m1", [P, NT * H], bf16, isOutput=False)
    m2_ext = nc.declare_dram_parameter("m2", [P, NT * H], bf16, isOutput=False)
    out_ext = nc.declare_dram_parameter("out", [NPC_PAD, HC], bf16,
                                        isOutput=True)
    shardbuf = nc.dram_tensor("shardbuf", [NSH, 72], bf16)
    gathered = nc.dram_tensor("gathered", [NXT, 72], bf16)
    xpb = nc.dram_tensor("xpb", [NXT, RW], bf16)
    # dma_gather mis-addresses offset source views, so window 2 gets its own
    # physical copy of rows [W2OFF, NXT)
    xpb2 = nc.dram_tensor("xpb2", [NXT - W2OFF, RW], bf16)

    with tile.TileContext(nc) as tc, ExitStack() as ctx:
        singles = ctx.enter_context(tc.tile_pool(name="singles", bufs=1))
        nc.gpsimd.load_library(library_config.mlp)
        wr_sb = singles.tile([IN, RW], bf16)
        nc.sync.dma_start(out=wr_sb[:], in_=wr_ext[:])
        bias_sb = singles.tile([P, HC], f32)
        nc.sync.dma_start(out=bias_sb[:], in_=bias_ext[:])
        ix1_sb = singles.tile([P, sw1[-1]], i16)
        ix2_sb = singles.tile([P, sw2[-1]], i16)
        for g in range(8):
            nc.sync.dma_start(out=ix1_sb[16 * g:16 * (g + 1), :],
                              in_=ix1_ext[:])
            nc.sync.dma_start(out=ix2_sb[16 * g:16 * (g + 1), :],
                              in_=ix2_ext[:])
        m1_sb = singles.tile([P, NT * H], bf16)
        nc.sync.dma_start(out=m1_sb[:], in_=m1_ext[:])
        m2_sb = singles.tile([P, NT * H], bf16)
        nc.sync.dma_start(out=m2_sb[:], in_=m2_ext[:])
        neg_sb = singles.tile([1, 4], bf16)
        nc.vector.memset(neg_sb[:], -1e30)

        # phase 1: project local shard -> [xp | a_src | a_dst] rows
        # (local newid g*512+b*128+p -> shardbuf row g*512+p*4+b)
        with ExitStack() as p1:
            xbufs = p1.enter_context(tc.tile_pool(name="xbufs", bufs=3))
            psums = p1.enter_context(tc.tile_pool(name="psums", bufs=4,
                                                  space="PSUM"))
            obufs = p1.enter_context(tc.tile_pool(name="obufs", bufs=3))
            for g in range(NGS if "noproj" not in _ABL else 0):
                c0 = g * 512
                xtile = xbufs.tile([IN, 512], bf16)
                nc.sync.dma_start(out=xtile[:], in_=xs_ext[:, c0:c0 + 512])
                ps = psums.tile([P, 4 * RW], f32, space="PSUM")
                for b in range(4):
                    nc.tensor.matmul(out=ps[:, b * RW:(b + 1) * RW],
                                     lhsT=xtile[:, b * P:(b + 1) * P],
                                     rhs=wr_sb[:], start=True, stop=True)
                xa = obufs.tile([P, 4 * RW], bf16)
                nc.scalar.copy(out=xa[:], in_=ps[:])
                xa3 = xa[:].rearrange("p (q w) -> p q w", q=4, w=RW)[:, :, 0:72]
                nc.sync.dma_start(
                    out=shardbuf[c0:c0 + 512, :].rearrange(
                        "(p q) w -> p q w", p=P, q=4),
                    in_=xa3)

        # all-gather shards -> full compact table, then spread to 256B rows
        nc.gpsimd.collective_compute(
            "AllGather", mybir.AluOpType.bypass,
            replica_groups=[list(range(NCORES))],
            ins=[shardbuf[:]], outs=[gathered[:]])
        nc.sync.dma_start(out=xpb[:, 0:72], in_=gathered[:])
        nc.sync.dma_start(out=xpb2[:, 0:72], in_=gathered[W2OFF:, :])
        nc.sync.dma_start(out=xpb[NEG1:NEG1 + 1, 64:68], in_=neg_sb[:])
        nc.sync.dma_start(out=xpb[NEG2:NEG2 + 1, 64:68], in_=neg_sb[:])
        nc.sync.dma_start(out=xpb2[NEG2 - W2OFF:NEG2 - W2OFF + 1, 64:68],
                          in_=neg_sb[:])

        # phase 2
        gath = ctx.enter_context(tc.tile_pool(name="gath", bufs=3))
        work = ctx.enter_context(tc.tile_pool(name="work", bufs=2))
        small = ctx.enter_context(tc.tile_pool(name="small", bufs=3))
        for t in range(NT):
            K1, K2 = k1s[t], k2s[t]
            E1, E2 = K1 - 1, K2 - 1   # edge slots (slot 0 = own row)
            D = E1 + E2
            r0 = t * P

            def chunked_gather(xg, src, isb, o0, k):
                for c0c in range(0, k, GCHUNK):
                    kc = min(GCHUNK, k - c0c)
                    nc.gpsimd.dma_gather(
                        xg[:, c0c * RW:(c0c + kc) * RW].rearrange(
                            "p (j w) -> p j w", j=kc, w=RW),
                        src[:], isb[:, o0 + 8 * c0c:o0 + 8 * (c0c + kc)],
                        kc * P, kc * P, RW)

            xg1 = gath.tile([P, K1 * RW], bf16)
            chunked_gather(xg1[:], xpb, ix1_sb, sw1[t], K1)
            xg2 = gath.tile([P, K2 * RW], bf16)
            chunked_gather(xg2[:], xpb2, ix2_sb, sw2[t], K2)
            x13 = xg1[:].rearrange("p (j w) -> p j w", j=K1, w=RW)
            x23 = xg2[:].rearrange("p (j w) -> p j w", j=K2, w=RW)

            if "nocompute" in _ABL:
                outsb0 = small.tile([P, HC], bf16)
                nc.vector.tensor_copy(out=outsb0[:], in_=xg1[:, :HC])
                nc.sync.dma_start(out=out_ext[r0:r0 + P, :], in_=outsb0[:])
                continue

            # a_dst of the tile's own nodes: slot 0 of whichever window
            adst = small.tile([P, H], f32)
            nc.vector.tensor_tensor(out=adst[:], in0=xg1[:, 68:72],
                                    in1=m1_sb[:, t * H:(t + 1) * H], op=mult)
            adst2t = small.tile([P, H], f32)
            nc.vector.tensor_tensor(out=adst2t[:], in0=xg2[:, 68:72],
                                    in1=m2_sb[:, t * H:(t + 1) * H], op=mult)
            nc.vector.tensor_tensor(out=adst[:], in0=adst[:], in1=adst2t[:],
                                    op=add)

            e = small.tile([P, D * H], f32)
            e3 = e[:].rearrange("p (d h) -> p d h", d=D, h=H)
            adst_b = adst[:].rearrange("p (one h) -> p one h", one=1)
            nc.vector.tensor_tensor(out=e3[:, :E1, :],
                                    in0=x13[:, 1:, 64:68],
                                    in1=adst_b.to_broadcast([P, E1, H]),
                                    op=add)
            nc.vector.tensor_tensor(out=e3[:, E1:, :],
                                    in0=x23[:, 1:, 64:68],
                                    in1=adst_b.to_broadcast([P, E2, H]),
                                    op=add)
            nc.vector.scalar_tensor_tensor(out=e[:], in0=e[:], scalar=0.2,
                                           in1=e[:], op0=mult,
                                           op1=mybir.AluOpType.max)
            exb = work.tile([P, D * HC], bf16)
            exb4 = exb[:].rearrange("p (d h c) -> p d h c", d=D, h=H, c=C)
            e4b = e[:].rearrange("p (d h one) -> p d h one", d=D, h=H,
                                 one=1).to_broadcast([P, D, H, C])
            nc.scalar.activation(out=exb4, in_=e4b,
                                 func=mybir.ActivationFunctionType.Exp)
            s = small.tile([P, H], f32)
            nc.vector.tensor_reduce(
                out=s[:],
                in_=exb[:].rearrange("p (d h c) -> p h c d", d=D, h=H,
                                     c=C)[:, :, 0:1, :],
                axis=X, op=add)
            sinv = small.tile([P, H], f32)
            nc.vector.reciprocal(out=sinv[:], in_=s[:])

            msg = work.tile([P, D * HC], bf16)
            msg3 = msg[:].rearrange("p (d w) -> p d w", d=D, w=HC)
            exb3 = exb[:].rearrange("p (d w) -> p d w", d=D, w=HC)
            nc.vector.tensor_tensor(out=msg3[:, :E1, :],
                                    in0=x13[:, 1:, 0:64],
                                    in1=exb3[:, :E1, :], op=mult)
            nc.vector.tensor_tensor(out=msg3[:, E1:, :],
                                    in0=x23[:, 1:, 0:64],
                                    in1=exb3[:, E1:, :], op=mult)
            n = D
            while n > 1:
                if n % 2:
                    nc.vector.tensor_tensor(out=msg3[:, n - 2, :],
                                            in0=msg3[:, n - 2, :],
                                            in1=msg3[:, n - 1, :], op=add)
                    n -= 1
                h = n // 2
                nc.vector.tensor_tensor(out=msg[:, :h * HC],
                                        in0=msg[:, :h * HC],
                                        in1=msg[:, h * HC:2 * h * HC], op=add)
                n = h

            outsb = small.tile([P, HC], f32)
            sinv_b = sinv[:].rearrange("p (h one) -> p h one",
                                       h=H, one=1).to_broadcast([P, H, C])
            nc.vector.tensor_tensor(
                out=outsb[:].rearrange("p (h c) -> p h c", h=H, c=C),
                in0=msg3[:, 0, :].rearrange("p (h c) -> p h c", h=H, c=C),
                in1=sinv_b, op=mult)
            nc.vector.tensor_tensor(out=outsb[:], in0=outsb[:], in1=bias_sb[:],
                                    op=add)
            outbf = small.tile([P, HC], bf16)
            nc.vector.tensor_copy(out=outbf[:], in_=outsb[:])
            nc.sync.dma_start(out=out_ext[r0:r0 + P, :], in_=outbf[:])

    nc.compile()
    return nc


def _trow(rank):
    """T-rank -> gather-table row (core-contiguous, phase-1 interleave)."""
    rank = np.asarray(rank)
    ct, l = rank // 6400, rank % 6400
    return ct * NSH + (l // 512) * 512 + (l % P) * 4 + ((l // P) % 4)


def _padrow(core, j):
    """Row of pad slot j (0..255) of a core (content = projected zeros)."""
    l = 6400 + j
    return core * NSH + (l // 512) * 512 + (l % P) * 4 + ((l // P) % 4)


ZERO1 = int(_padrow(0, 0))
NEG1 = int(_padrow(0, 1))
ZERO2 = int(_padrow(7, 0))
NEG2 = int(_padrow(7, 1))
assert ZERO1 < W1LIM and NEG1 < W1LIM
assert ZERO2 >= W2OFF and NEG2 >= W2OFF and NEG2 - W2OFF < 32768


def _wrap16(vals, k):
    """vals[p, d] -> unreplicated int16 [16, 8k]: unwrapped[i]=w[i%16,i//16],
    i = d*128 + p (device replicates rows 0:16 to all 8 groups)."""
    i = np.arange(P * k)
    w = np.zeros((16, 8 * k), np.int16)
    w[i % 16, i // 16] = vals[i % P, i // P]
    return w


def _preprocess(edge_index):
    src = edge_index[0].astype(np.int64)
    dst = edge_index[1].astype(np.int64)
    deg = np.bincount(dst, minlength=N)

    # T mapping: rank = identity order, core-contiguous blocks of 6400
    orderT = np.arange(N)
    trank_of_node = np.arange(N)
    trow_of_node = _trow(trank_of_node)

    srcr = trow_of_node[src]

    # per-dst window counts over source rows
    n1 = np.zeros(N, np.int64)
    n2 = np.zeros(N, np.int64)
    m = np.zeros(N, np.int64)
    c1 = srcr < W2OFF
    c2 = srcr >= W1LIM
    np.add.at(n1, dst[c1], 1)
    np.add.at(n2, dst[c2], 1)
    np.add.at(m, dst[~c1 & ~c2], 1)
    a = np.clip((n2 + m - n1 + 1) // 2, 0, m)   # overlap going to window 1
    A = n1 + a + 1 + (deg == 0)   # + own slot (+ zero-dummy for empty nodes)
    B = n2 + m - a + 1

    # S mapping (dst shard slots): sort by (A, B) desc -> uniform tiles;
    # snake B within A-groups so tile-range B-maxes stay tight at boundaries
    orderS = np.lexsort((B, A))[::-1]
    As = A[orderS]
    gid = np.cumsum(np.r_[True, As[1:] != As[:-1]]) - 1
    for g in range(gid.max() + 1):
        if g % 2 == 1:
            sel = np.where(gid == g)[0]
            orderS[sel] = orderS[sel[::-1]]
    r = np.arange(NX)
    k, p = r // P, r % P
    snewid_of_rank = (k % NCORES) * NPC_PAD + (k // NCORES) * P + p
    snewid_of_node = np.empty(N, np.int64)
    snewid_of_node[orderS] = snewid_of_rank[:N]
    node_of_snewid = np.full(NX, -1, np.int64)
    node_of_snewid[snewid_of_rank[:N]] = orderS

    # per-tile K maxes, shared across cores for the SPMD program
    tile_of = (np.arange(NX) % NPC_PAD) // P
    Av = np.zeros(NX, np.int64)
    Bv = np.zeros(NX, np.int64)
    Av[snewid_of_node] = A
    Bv[snewid_of_node] = B
    k1s, k2s = [], []
    for t in range(NT):
        sel = tile_of == t
        k1s.append(max(2, int(Av[sel].max())))
        k2s.append(max(2, int(Bv[sel].max())))

    # slot tables indexed by snewid; slot 0 = own row (or dummy)
    ix1 = np.full((NX, max(k1s)), NEG1, np.int32)
    ix2 = np.full((NX, max(k2s)), NEG2 - W2OFF, np.int32)
    sn = snewid_of_node
    own_r = trow_of_node
    own_w1 = own_r < W1LIM
    ix1[sn[own_w1], 0] = own_r[own_w1]
    ix2[sn[~own_w1], 0] = own_r[~own_w1] - W2OFF
    m1 = np.zeros(NX, np.float32)
    m1[sn[own_w1]] = 1.0

    # order each node's edges: window-1 = n1 edges then a of overlap
    dstn = sn[dst]
    osort = np.argsort(dstn, kind="stable")
    dstn_s, srcr_s = dstn[osort], srcr[osort]
    cls = np.where(c1[osort], 0,
                   np.where(c2[osort], 2, 1))  # 0=w1only, 1=overlap, 2=w2only
    keys = dstn_s * 4 + cls
    ordc = np.argsort(keys, kind="stable")
    keys_s = keys[ordc]
    first = np.r_[True, keys_s[1:] != keys_s[:-1]]
    grp_start = np.zeros(len(keys_s), np.int64)
    grp_start[first] = np.arange(len(keys_s))[first]
    grp_start = np.maximum.accumulate(grp_start)
    pos_in_cls = np.arange(len(keys_s)) - grp_start
    posc = np.empty(E, np.int64)
    posc[ordc] = pos_in_cls
    dn = dstn_s
    node_dn = node_of_snewid[dn]
    av = a[node_dn]
    n1v = n1[node_dn]
    mv = m[node_dn]
    # window-1 slots (1-based, after own): w1only edges then first av overlap
    w1_mask = (cls == 0) | ((cls == 1) & (posc < av))
    slot_w1 = 1 + np.where(cls == 0, posc, n1v + posc)
    ix1[dn[w1_mask], slot_w1[w1_mask]] = srcr_s[w1_mask]
    ovr = (cls == 1) & (posc >= av)
    ix2[dn[ovr], 1 + posc[ovr] - av[ovr]] = srcr_s[ovr] - W2OFF
    mw2 = cls == 2
    ix2[dn[mw2], 1 + (mv - av)[mw2] + posc[mw2]] = srcr_s[mw2] - W2OFF
    # empty nodes: one zero-dummy edge slot in window 1
    empty_sn = sn[deg == 0]
    ix1[empty_sn, 1] = ZERO1

    return (k1s, k2s, ix1, ix2, m1, orderT, node_of_snewid)


def _prepare(x, edge_index, W, att_src, att_dst, bias):
    x = np.asarray(x, np.float32)
    edge_index = np.asarray(edge_index)
    W = np.asarray(W, np.float32)
    att_src = np.asarray(att_src, np.float32)
    att_dst = np.asarray(att_dst, np.float32)
    bias = np.asarray(bias, np.float32)

    (k1s, k2s, ix1, ix2, m1, orderT, node_of_snewid) = _preprocess(edge_index)
    key = (tuple(k1s), tuple(k2s))
    if key not in _cache:
        _cache[key] = build_program(k1s, k2s)
    nc = _cache[key]

    wa_src = np.einsum("ihc,hc->ih", W.T.reshape(IN, H, C), att_src)
    wa_dst = np.einsum("ihc,hc->ih", W.T.reshape(IN, H, C), att_dst)
    wrhs = np.zeros((IN, RW), np.float32)
    wrhs[:, :HC] = W.T
    wrhs[:, HC:HC + H] = wa_src
    wrhs[:, HC + H:HC + 2 * H] = wa_dst
    bias_rep = np.tile(bias.reshape(1, HC), (P, 1)).astype(np.float32)

    import ml_dtypes

    def bf16(arr):
        return arr.astype(ml_dtypes.bfloat16)

    xT = np.ascontiguousarray(x.T)  # [IN, N], T order = identity
    in_maps = []
    for c in range(NCORES):
        lo = c * NPC_PAD
        w1 = [_wrap16(ix1[lo + t * P:lo + (t + 1) * P, :k1s[t]]
                      .astype(np.int16), k1s[t]) for t in range(NT)]
        w2 = [_wrap16(ix2[lo + t * P:lo + (t + 1) * P, :k2s[t]]
                      .astype(np.int16), k2s[t]) for t in range(NT)]
        m1c = m1[lo:lo + NPC_PAD].reshape(NT, P).T          # [P, NT]
        m1h = np.repeat(m1c[:, :, None], H, axis=2).reshape(P, NT * H)
        hi = min((c + 1) * 6400, N)
        xs = np.zeros((IN, NSH), np.float32)
        xs[:, :hi - c * 6400] = xT[:, c * 6400:hi]
        in_maps.append({
            "xs": bf16(xs), "wrhs": bf16(wrhs), "bias": bias_rep,
            "ix1": np.concatenate(w1, axis=1),
            "ix2": np.concatenate(w2, axis=1),
            "m1": bf16(m1h), "m2": bf16(1.0 - m1h),
        })

    def post(res):
        arr = np.concatenate([np.asarray(res[c]["out"], np.float32)
                              for c in range(NCORES)], axis=0)
        valid = node_of_snewid >= 0
        out = np.zeros((N, HC), np.float32)
        out[node_of_snewid[valid]] = arr[valid]
        return out

    return nc, in_maps, post


def kernel(x, edge_index, W, att_src, att_dst, bias):
    nc, in_maps, post = _prepare(x, edge_index, W, att_src, att_dst, bias)
    global _last_results
    tmpdir = None
    if os.environ.get("BASS_TRACE"):
        tmpdir = tempfile.mkdtemp(prefix="gat_trace_")
    _last_results = run_bass_kernel_spmd(nc, in_maps, list(range(NCORES)),
                                         tmpdir=tmpdir)
    return post(_last_results.results)
